# revision 2
# baseline (speedup 1.0000x reference)
"""Single-level 2D Haar DWT (periodization mode) on Trainium2.

Input x: (8, 512, 512, 16) fp32 NHWC. Output: (LL, LH, HL, HH), each
(8, 256, 256, 16) fp32 — +/- combinations of each 2x2 spatial block,
scaled by 0.5.

Sharding: pure data parallel — one batch sample per NeuronCore (8 cores).

Per-core kernel structure (x viewed as (512, 8192) row-major):
  - 2 pair-chunks (128 row-pairs) x 4 W-blocks = 8 iterations
  - per iter: one 2 MB DMA loads [128 pairs, 2 rows, 2048] into SBUF
    (contiguous 8 KB runs per partition),
    8 tensor_add/sub ops implement the 2x2 butterfly (strided SBUF reads)
    split across DVE and GpSimd, 4 in-place x0.5 scales on ACT,
    then 4x 512 KB contiguous output DMAs.
"""

import sys

if "/opt/trn_rl_repo" not in sys.path:
    sys.path.insert(0, "/opt/trn_rl_repo")

import numpy as np

B, H, W, C = 8, 512, 512, 16
N_CORES = 8
HO, WO = H // 2, W // 2  # 256, 256
ROW = W * C  # 8192 elements per input row
OROW = WO * C  # 4096 elements per output row

_CACHE = {}


def _build():
    import concourse.bacc as bacc
    import concourse.mybir as mybir
    import concourse.tile as tile

    fp32 = mybir.dt.float32

    nc = bacc.Bacc(
        "TRN2", target_bir_lowering=False, debug=False, num_devices=N_CORES
    )
    x = nc.dram_tensor("x", (H, ROW), fp32, kind="ExternalInput")
    outs = {
        name: nc.dram_tensor(name, (HO, WO, C), fp32, kind="ExternalOutput")
        for name in ("LL", "LH", "HL", "HH")
    }

    # x rows grouped into pairs: [pair q, t in {0,1}, row elems]
    xq = x.rearrange("(q t) m -> q t m", t=2)

    PAIRS = 128  # row-pairs per iteration (partition dim)
    WBLK = ROW // 4  # 2048 input elements per W-block per row
    WQ = WBLK // (2 * C)  # 64 W-pairs per block

    with tile.TileContext(nc) as tc:
        with (
            tc.tile_pool(name="inp", bufs=3) as inp,
            tc.tile_pool(name="mid", bufs=3) as mid,
            tc.tile_pool(name="outp", bufs=3) as outp,
        ):
            it = 0
            for pc in range(H // 2 // PAIRS):  # 2 pair-chunks
                for wb in range(ROW // WBLK):  # 4 W-blocks
                    xt = inp.tile([PAIRS, 2, WBLK], fp32)
                    nc.sync.dma_start(
                        xt[:],
                        xq[
                            pc * PAIRS : (pc + 1) * PAIRS,
                            :,
                            wb * WBLK : (wb + 1) * WBLK,
                        ],
                    )
                    # [pair, t, wq, u, c] view: t = row parity, u = col parity
                    xv = xt[:].rearrange("p t (w u c) -> p t w u c", u=2, c=C)
                    a = xv[:, 0, :, 0, :]
                    b = xv[:, 0, :, 1, :]
                    c_ = xv[:, 1, :, 0, :]
                    d = xv[:, 1, :, 1, :]

                    t1 = mid.tile([PAIRS, WQ, C], fp32, tag="t1")
                    t2 = mid.tile([PAIRS, WQ, C], fp32, tag="t2")
                    u1 = mid.tile([PAIRS, WQ, C], fp32, tag="u1")
                    u2 = mid.tile([PAIRS, WQ, C], fp32, tag="u2")
                    # GpSimd TT is ~2x slower than DVE; balance ~2:1 by op
                    # count (DVE 5-6, GpSimd 2-3 per iteration).
                    gp = nc.gpsimd
                    ve = nc.vector
                    e1 = gp if it % 2 == 0 else ve  # alternates the 6th op
                    ve.tensor_add(t1[:], a, b)
                    gp.tensor_add(t2[:], c_, d)
                    ve.tensor_sub(u1[:], a, b)
                    e1.tensor_sub(u2[:], c_, d)

                    res = {}
                    for name, i0, i1, eng, op in (
                        ("LL", t1, t2, ve, "add"),
                        ("HL", t1, t2, ve, "sub"),
                        ("LH", u1, u2, gp, "add"),
                        ("HH", u1, u2, ve, "sub"),
                    ):
                        ot = outp.tile([PAIRS, WQ, C], fp32, tag=name)
                        if op == "add":
                            eng.tensor_add(ot[:], i0[:], i1[:])
                        else:
                            eng.tensor_sub(ot[:], i0[:], i1[:])
                        nc.any.tensor_scalar_mul(ot[:], ot[:], 0.5)
                        res[name] = ot

                    for name, ot in res.items():
                        nc.sync.dma_start(
                            outs[name][
                                pc * PAIRS : (pc + 1) * PAIRS,
                                wb * WQ : (wb + 1) * WQ,
                                :,
                            ],
                            ot[:],
                        )
                    it += 1

    nc.compile()
    return nc


def _get_nc():
    if "nc" not in _CACHE:
        _CACHE["nc"] = _build()
    return _CACHE["nc"]


def kernel(x):
    from concourse.bass_utils import run_bass_kernel_spmd

    x = np.asarray(x, dtype=np.float32)
    assert x.shape == (B, H, W, C), x.shape

    nc = _get_nc()
    in_maps = [{"x": np.ascontiguousarray(x[i].reshape(H, ROW))} for i in range(B)]
    res = run_bass_kernel_spmd(nc, in_maps, list(range(N_CORES)))

    out = []
    for name in ("LL", "LH", "HL", "HH"):
        out.append(np.stack([res.results[i][name] for i in range(B)], axis=0))
    return tuple(out)


# revision 4
# speedup vs baseline: 1.0018x; 1.0018x over previous
"""Single-level 2D Haar DWT (periodization mode) on Trainium2.

Input x: (8, 512, 512, 16) fp32 NHWC. Output: (LL, LH, HL, HH), each
(8, 256, 256, 16) fp32 — +/- combinations of each 2x2 spatial block,
scaled by 0.5.

Sharding: pure data parallel — one batch sample per NeuronCore (8 cores).

Per-core kernel (x viewed as (512, 8192) row-major):
  - TensorE does the row-direction (H) butterfly as a matmul with a
    fixed 128x128 weight holding +/-0.5: out rows 0..63 = 0.5*(top+bot)
    per row-pair, rows 64..127 = 0.5*(top-bot). The 0.5 subband scale is
    folded into the weights.
  - DVE does the column-direction (W) butterfly straight out of PSUM:
    even +/- odd W position -> (LL|HL) and (LH|HH) SBUF tiles.
  - 4 K-chunks of 128 input rows; per chunk one 4 MB fully-contiguous
    input DMA and 4x 1 MB fully-contiguous output DMAs.
"""

import sys

if "/opt/trn_rl_repo" not in sys.path:
    sys.path.insert(0, "/opt/trn_rl_repo")

import numpy as np

B, H, W, C = 8, 512, 512, 16
N_CORES = 8
HO, WO = H // 2, W // 2  # 256, 256
ROW = W * C  # 8192 elements per input row
OROW = WO * C  # 4096 elements per output row

_CACHE = {}


def _haar_weight():
    """lhsT [k, m]: matmul computes out[m, n] = sum_k w[k, m] x[k, n].

    m in 0..63:   0.5*(row 2m + row 2m+1)   (sum rows)
    m in 64..127: 0.5*(row 2m' - row 2m'+1), m' = m - 64  (diff rows)
    """
    w = np.zeros((128, 128), dtype=np.float32)
    for m in range(64):
        w[2 * m, m] = 0.5
        w[2 * m + 1, m] = 0.5
        w[2 * m, 64 + m] = 0.5
        w[2 * m + 1, 64 + m] = -0.5
    return w


def _build():
    import concourse.bacc as bacc
    import concourse.mybir as mybir
    import concourse.tile as tile

    fp32 = mybir.dt.float32

    nc = bacc.Bacc(
        "TRN2", target_bir_lowering=False, debug=False, num_devices=N_CORES
    )
    x = nc.dram_tensor("x", (H, ROW), fp32, kind="ExternalInput")
    wdram = nc.dram_tensor("w", (128, 128), fp32, kind="ExternalInput")
    outs = {
        name: nc.dram_tensor(name, (HO, WO, C), fp32, kind="ExternalOutput")
        for name in ("LL", "LH", "HL", "HH")
    }

    KC = H // 128  # 4 K-chunks of 128 input rows (64 row-pairs each)
    NG = 4  # PSUM groups per K-chunk
    GN = ROW // NG  # 2048 input cols per group (4 PSUM banks)
    MM_N = 512  # moving free dim per matmul (one PSUM bank, fp32)

    with tile.TileContext(nc) as tc:
        with (
            tc.tile_pool(name="wpool", bufs=1) as wpool,
            tc.tile_pool(name="inp", bufs=2) as inp,
            tc.tile_pool(name="psum", bufs=2, space="PSUM") as psum,
            tc.tile_pool(name="sbp", bufs=2) as sbp,
            tc.tile_pool(name="outp", bufs=2) as outp,
        ):
            wt = wpool.tile([128, 128], fp32)
            nc.sync.dma_start(wt[:], wdram[:])

            for kc in range(KC):
                xt = inp.tile([128, ROW], fp32)
                nc.sync.dma_start(xt[:], x[kc * 128 : (kc + 1) * 128, :])

                # rows 0..63 = LL|LH source pairs, 64..127 = HL|HH
                sum_t = outp.tile([128, OROW], fp32, tag="sum")
                diff_t = outp.tile([128, OROW], fp32, tag="diff")

                for sg in range(NG // 2):  # 2 super-groups of 2 PSUM tiles
                    sb = sbp.tile([128, 2 * GN], fp32)
                    for h in range(2):
                        g = 2 * sg + h
                        ps = psum.tile([128, GN], fp32)
                        for j in range(GN // MM_N):
                            lo = j * MM_N
                            nc.tensor.matmul(
                                ps[:, lo : lo + MM_N],
                                wt[:],
                                xt[:, g * GN + lo : g * GN + lo + MM_N],
                                start=True,
                                stop=True,
                            )
                        # PSUM -> SBUF on ScalarE (ACT), freeing DVE
                        nc.scalar.copy(
                            sb[:, h * GN : (h + 1) * GN], ps[:]
                        )
                    # even/odd W combine on DVE, all-SBUF: [p, wq, u, c]
                    sv_in = sb[:].rearrange("p (w u c) -> p w u c", u=2, c=C)
                    ev = sv_in[:, :, 0, :]
                    od = sv_in[:, :, 1, :]
                    go = sg * GN
                    sv = sum_t[:, go : go + GN].rearrange(
                        "p (w c) -> p w c", c=C
                    )
                    dv = diff_t[:, go : go + GN].rearrange(
                        "p (w c) -> p w c", c=C
                    )
                    nc.vector.tensor_add(sv, ev, od)
                    nc.vector.tensor_sub(dv, ev, od)

                r0, r1 = kc * 64, (kc + 1) * 64
                nc.sync.dma_start(
                    outs["LL"][r0:r1].rearrange("h w c -> h (w c)"),
                    sum_t[0:64, :],
                )
                nc.sync.dma_start(
                    outs["HL"][r0:r1].rearrange("h w c -> h (w c)"),
                    sum_t[64:128, :],
                )
                nc.sync.dma_start(
                    outs["LH"][r0:r1].rearrange("h w c -> h (w c)"),
                    diff_t[0:64, :],
                )
                nc.sync.dma_start(
                    outs["HH"][r0:r1].rearrange("h w c -> h (w c)"),
                    diff_t[64:128, :],
                )

    nc.compile()
    return nc


def _get_nc():
    if "nc" not in _CACHE:
        _CACHE["nc"] = _build()
    return _CACHE["nc"]


def kernel(x):
    from concourse.bass_utils import run_bass_kernel_spmd

    x = np.asarray(x, dtype=np.float32)
    assert x.shape == (B, H, W, C), x.shape

    nc = _get_nc()
    w = _haar_weight()
    in_maps = [
        {"x": np.ascontiguousarray(x[i].reshape(H, ROW)), "w": w}
        for i in range(B)
    ]
    res = run_bass_kernel_spmd(nc, in_maps, list(range(N_CORES)))

    out = []
    for name in ("LL", "LH", "HL", "HH"):
        out.append(np.stack([res.results[i][name] for i in range(B)], axis=0))
    return tuple(out)


# revision 5
# speedup vs baseline: 1.1982x; 1.1961x over previous
"""Single-level 2D Haar DWT (periodization mode) on Trainium2.

Input x: (8, 512, 512, 16) fp32 NHWC. Output: (LL, LH, HL, HH), each
(8, 256, 256, 16) fp32 — +/- combinations of each 2x2 spatial block,
scaled by 0.5.

Sharding: pure data parallel — one batch sample per NeuronCore (8 cores).

Per-core kernel (x viewed as (512, 8192) row-major), 4 iterations of
(pair-chunk in {0,1}) x (W-half in {0,1}):
  - two 2 MB DMAs load even rows ("top") and odd rows ("bot") of 128
    row-pairs x half-W into separate SBUF tiles (16 KB contiguous runs),
    issued on the SP HWDGE ring.
  - 8 tensor_add/sub ops of [128, 2048] implement the 2x2 butterfly:
    DVE takes 6, GpSimd takes 2 (top/bot streams are independent so the
    engines run concurrently); ACT applies the x0.5 scale in place.
  - 4x 1 MB contiguous output DMAs per iteration on the ACT HWDGE ring
    (separate descriptor FIFO so stores never delay the next load).
"""

import sys

if "/opt/trn_rl_repo" not in sys.path:
    sys.path.insert(0, "/opt/trn_rl_repo")

import numpy as np

B, H, W, C = 8, 512, 512, 16
N_CORES = 8
HO, WO = H // 2, W // 2  # 256, 256
ROW = W * C  # 8192 elements per input row
OROW = WO * C  # 4096 elements per output row

_CACHE = {}


def _build():
    import concourse.bacc as bacc
    import concourse.mybir as mybir
    import concourse.tile as tile

    fp32 = mybir.dt.float32

    nc = bacc.Bacc(
        "TRN2", target_bir_lowering=False, debug=False, num_devices=N_CORES
    )
    x = nc.dram_tensor("x", (H, ROW), fp32, kind="ExternalInput")
    outs = {
        name: nc.dram_tensor(name, (HO, WO, C), fp32, kind="ExternalOutput")
        for name in ("LL", "LH", "HL", "HH")
    }

    # x rows grouped into pairs: [pair q, t in {0,1}, row elems]
    xq = x.rearrange("(q t) m -> q t m", t=2)

    PAIRS = 128  # row-pairs per iteration (partition dim)
    WBLK = ROW // 2  # 4096 input elements per W-half per row
    WQ = WBLK // (2 * C)  # 128 W-pairs per half

    with tile.TileContext(nc) as tc:
        with (
            tc.tile_pool(name="inp", bufs=2) as inp,
            tc.tile_pool(name="mid", bufs=2) as mid,
            tc.tile_pool(name="outp", bufs=2) as outp,
        ):
            for pc in range(H // 2 // PAIRS):  # 2 pair-chunks
                for wb in range(ROW // WBLK):  # 2 W-halves
                    top = inp.tile([PAIRS, WBLK], fp32, tag="top")
                    bot = inp.tile([PAIRS, WBLK], fp32, tag="bot")
                    qs = slice(pc * PAIRS, (pc + 1) * PAIRS)
                    ws = slice(wb * WBLK, (wb + 1) * WBLK)
                    nc.sync.dma_start(top[:], xq[qs, 0, ws])
                    nc.sync.dma_start(bot[:], xq[qs, 1, ws])

                    # even/odd W views: [pair, wq, u, c]
                    tv = top[:].rearrange("p (w u c) -> p w u c", u=2, c=C)
                    bv = bot[:].rearrange("p (w u c) -> p w u c", u=2, c=C)
                    a, b = tv[:, :, 0, :], tv[:, :, 1, :]
                    c_, d = bv[:, :, 0, :], bv[:, :, 1, :]

                    t1 = mid.tile([PAIRS, WQ, C], fp32, tag="t1")
                    t2 = mid.tile([PAIRS, WQ, C], fp32, tag="t2")
                    u1 = mid.tile([PAIRS, WQ, C], fp32, tag="u1")
                    u2 = mid.tile([PAIRS, WQ, C], fp32, tag="u2")
                    # GpSimd handles the bot-row stream, DVE the top-row
                    # stream and all of stage 2 (GpSimd TT is ~2x slower).
                    nc.gpsimd.tensor_add(t2[:], c_, d)
                    nc.gpsimd.tensor_sub(u2[:], c_, d)
                    nc.vector.tensor_add(t1[:], a, b)
                    nc.vector.tensor_sub(u1[:], a, b)

                    res = {}
                    for name, i0, i1, op in (
                        ("LL", t1, t2, "add"),
                        ("HL", t1, t2, "sub"),
                        ("LH", u1, u2, "add"),
                        ("HH", u1, u2, "sub"),
                    ):
                        ot = outp.tile([PAIRS, WQ, C], fp32, tag=name)
                        if op == "add":
                            nc.vector.tensor_add(ot[:], i0[:], i1[:])
                        else:
                            nc.vector.tensor_sub(ot[:], i0[:], i1[:])
                        nc.scalar.mul(ot[:], ot[:], 0.5)
                        res[name] = ot

                    for name, ot in res.items():
                        nc.scalar.dma_start(
                            outs[name][qs, wb * WQ : (wb + 1) * WQ, :],
                            ot[:],
                        )

    nc.compile()
    return nc


def _get_nc():
    if "nc" not in _CACHE:
        _CACHE["nc"] = _build()
    return _CACHE["nc"]


def kernel(x):
    from concourse.bass_utils import run_bass_kernel_spmd

    x = np.asarray(x, dtype=np.float32)
    assert x.shape == (B, H, W, C), x.shape

    nc = _get_nc()
    in_maps = [{"x": np.ascontiguousarray(x[i].reshape(H, ROW))} for i in range(B)]
    res = run_bass_kernel_spmd(nc, in_maps, list(range(N_CORES)))

    out = []
    for name in ("LL", "LH", "HL", "HH"):
        out.append(np.stack([res.results[i][name] for i in range(B)], axis=0))
    return tuple(out)
